# revision 8
# baseline (speedup 1.0000x reference)
"""Trajectory attention (2-stage spatial/temporal) on 8 Trainium2 cores.

Sharding: data-parallel over the 2 clips x 4-way sequence-parallel over each
clip's 1568 tokens (392 query tokens per core).  k/v are computed for the
whole clip on every core (replicated, ~3.7 GFLOP) which removes every
collective: cores are fully independent and the host gathers the shards.

Per-core frame permutation (own 2 frames first) keeps the single SPMD
program uniform: every core's query tokens are perm-tokens [0, 392) and the
stage-2 diagonal gather offsets become core-independent.
"""

import numpy as np
from contextlib import ExitStack

import concourse.bass as bass
import concourse.mybir as mybir
from concourse import masks
from concourse.tile import TileContext
from concourse.vector_clock import ScopedClock

# ---------------------------------------------------------------- constants
B, FR, P, C, H = 2, 8, 196, 768, 12
D = C // H                      # 64
N = FR * P                      # 1568 tokens per clip
SQ = N // 4                     # 392 query tokens per core
CH = 98                         # uniform partition chunk (1568 = 16*98)
NT = N // CH                    # 16 token chunks
SCALE = D ** -0.5
F32 = mybir.dt.float32
F32R = mybir.dt.float32r
EXP = mybir.ActivationFunctionType.Exp

USE_F32R = False                 # tf32 matmul path (4x faster than fp32)


def _mm_cast(ap):
    return ap.bitcast(F32R) if USE_F32R else ap


# ------------------------------------------------------- tile drain bugfix
# walrus CoreV3 rejects CTRL-queue instructions carrying >1 semaphore wait;
# TileContext's end-of-kernel drain can accumulate many.  Spread the waits
# over extra nops (same engine, before the barrier -> semantics unchanged).
def _patched_drain_and_barrier(self, tick_clock, wait_clock):
    nc = self.nc
    drain_inst = nc.sync.drain()
    wait_clock.add_sem_waits(
        drain_inst.ins, ScopedClock({None: tick_clock.global_clock})
    )
    si = drain_inst.ins.sync_info
    if si is not None and si.on_wait and len(si.on_wait) > 1:
        waits = list(si.on_wait)
        si.on_wait.clear()
        si.on_wait.extend(waits[:1])
        rest = waits[1:]
        while rest:
            nop = nc.sync.nop()
            nsi = nop.ins.sync_info
            if nsi is None:
                nop.ins.sync_info = mybir.SyncInfo(on_wait=[], on_update=[])
                nsi = nop.ins.sync_info
            nsi.on_wait.extend(rest[:1])
            rest = rest[1:]
    nc.all_engine_barrier()
    assert self.sems is not None
    popped = nc._tile_sem_poison_stack.pop()
    assert popped is self._sem_poison
    nc.clear_and_free_semaphores(list(self.sems.allocated().values()))
    nc.all_engine_barrier()


TileContext._drain_and_barrier = _patched_drain_and_barrier


def _split_excess_waits(nc):
    """walrus allows 1 sync wait per instruction; hoist extras onto nops."""
    ctr = 0
    for fn in nc.m.functions:
        for bb in fn.blocks:
            insts = list(bb.instructions)
            out = []
            changed = False
            for inst in insts:
                si = inst.sync_info
                if si is not None and si.on_wait and len(si.on_wait) > 1:
                    waits = list(si.on_wait)
                    for w in waits[:-1]:
                        nop = mybir.InstNoOp(
                            name=f"I-wsplit-{ctr}", ins=[], outs=[])
                        ctr += 1
                        nop.engine = inst.engine
                        nop.sync_info = mybir.SyncInfo(on_wait=[w], on_update=[])
                        out.append(nop)
                    si.on_wait.clear()
                    si.on_wait.append(waits[-1])
                    changed = True
                out.append(inst)
            if changed:
                bb.instructions = out


# ---------------------------------------------------------------- program
def _build():
    nc = bass.Bass()

    x_in = nc.declare_dram_parameter("x", [N, C], F32, isOutput=False)
    wqkvT = nc.declare_dram_parameter("wqkvT", [C, 3 * C], F32, isOutput=False)
    wqT = nc.declare_dram_parameter("wqT", [C, C], F32, isOutput=False)
    wkvT = nc.declare_dram_parameter("wkvT", [C, 2 * C], F32, isOutput=False)
    wprojT = nc.declare_dram_parameter("wprojT", [C, C], F32, isOutput=False)
    bproj = nc.declare_dram_parameter("bproj", [1, C], F32, isOutput=False)

    q_nat = nc.declare_dram_parameter("q_nat", [SQ, C], F32, isOutput=True)
    k_nat = nc.declare_dram_parameter("k_nat", [SQ, C], F32, isOutput=True)
    v_nat = nc.declare_dram_parameter("v_nat", [SQ, C], F32, isOutput=True)
    attn_o = nc.declare_dram_parameter("attn_o", [SQ, H * FR], F32, isOutput=True)
    out_o = nc.declare_dram_parameter("out_o", [SQ, C], F32, isOutput=True)

    copy_rr = [nc.vector.tensor_copy, nc.scalar.copy]

    def pcopy(i, dst, src):
        copy_rr[i % 2](dst, src)

    with TileContext(nc) as tc, ExitStack() as es0:
        const = es0.enter_context(tc.tile_pool(name="const", bufs=1))
        ident = const.tile([128, 128], F32, tag="ident", name="ident")
        masks.make_identity(nc, ident[:])
        ones_t = const.tile([CH, 128], F32, tag="ones", name="ones")
        nc.gpsimd.memset(ones_t[:], 1.0)
        bias_sb = const.tile([1, C], F32, tag="bias", name="bias")
        nc.sync.dma_start(bias_sb[:], bproj[:])

        es_bc = ExitStack()                   # lives through phases A..C
        kvq_pool = es_bc.enter_context(tc.tile_pool(name="kvq", bufs=1))
        kT = [kvq_pool.tile([128, N], F32, tag=f"kT{j}", name=f"kT{j}")
              for j in range(6)]
        qT = [kvq_pool.tile([128, SQ], F32, tag=f"qT{j}", name=f"qT{j}")
              for j in range(6)]
        v_t = [kvq_pool.tile([CH, C], F32, tag=f"v{i}", name=f"v{i}")
               for i in range(NT)]

        # ---------------- phase A+B: x transpose, qkv projections ----------
        with tc.tile_pool(name="xT", bufs=1) as xT_pool, \
             tc.tile_pool(name="xrow", bufs=3) as xrow_pool, \
             tc.tile_pool(name="wB", bufs=1) as wB_pool, \
             tc.tile_pool(name="stg", bufs=4) as stg_pool, \
             tc.tile_pool(name="psAB", bufs=1, space="PSUM") as psAB:
            xT = [xT_pool.tile([128, N], F32, tag=f"xT{j}", name=f"xT{j}")
                  for j in range(6)]
            # A: transpose x into xT
            for i in range(NT):
                xrow = xrow_pool.tile([CH, C], F32, tag="xrow", name="xrow")
                nc.sync.dma_start(xrow[:], x_in[i * CH:(i + 1) * CH, :])
                for ci in range(6):
                    ps = psAB.tile([128, CH], F32, tag="psA", name="psA", bufs=3)
                    nc.tensor.transpose(
                        ps[:], xrow[:, ci * 128:(ci + 1) * 128], ident[:CH, :CH])
                    pcopy(ci, xT[ci][:, i * CH:(i + 1) * CH], ps[:])

            # B1: kT (couts C..2C of wqkvT)
            wk = [wB_pool.tile([128, C], F32, tag=f"wk{ci}", name=f"wk{ci}")
                  for ci in range(6)]
            for ci in range(6):
                nc.sync.dma_start(wk[ci][:], wqkvT[ci * 128:(ci + 1) * 128, C:2 * C])
            for hp in range(6):
                for ns in range(4):
                    ps = psAB.tile([128, SQ], F32, tag="psB", name="psB", bufs=3)
                    for ci in range(6):
                        nc.tensor.matmul(
                            ps[:], _mm_cast(wk[ci][:, hp * 128:(hp + 1) * 128]),
                            _mm_cast(xT[ci][:, ns * SQ:(ns + 1) * SQ]),
                            start=ci == 0, stop=ci == 5)
                    pcopy(hp + ns, kT[hp][:, ns * SQ:(ns + 1) * SQ], ps[:])
            # B2: qT (couts 0..C), own tokens only
            for ci in range(6):
                nc.sync.dma_start(wk[ci][:], wqkvT[ci * 128:(ci + 1) * 128, 0:C])
            for hp in range(6):
                ps = psAB.tile([128, SQ], F32, tag="psB", name="psB", bufs=3)
                for ci in range(6):
                    nc.tensor.matmul(
                        ps[:], _mm_cast(wk[ci][:, hp * 128:(hp + 1) * 128]),
                        _mm_cast(xT[ci][:, 0:SQ]),
                        start=ci == 0, stop=ci == 5)
                pcopy(hp, qT[hp][:], ps[:])
            # B3: v natural
            for ci in range(6):
                nc.sync.dma_start(
                    wk[ci][:], wqkvT[ci * 128:(ci + 1) * 128, 2 * C:3 * C])
            for i in range(NT):
                for ns in range(2):
                    ps = psAB.tile([CH, 384], F32, tag="psBv", name="psBv", bufs=2)
                    for ci in range(6):
                        nc.tensor.matmul(
                            ps[:], _mm_cast(xT[ci][:, i * CH:(i + 1) * CH]),
                            _mm_cast(wk[ci][:, ns * 384:(ns + 1) * 384]),
                            start=ci == 0, stop=ci == 5)
                    pcopy(i + ns, v_t[i][:, ns * 384:(ns + 1) * 384], ps[:])

            # B4: q_nat / k_nat / v_nat outputs
            for i in range(4):
                nc.sync.dma_start(v_nat[i * CH:(i + 1) * CH, :], v_t[i][:])
            for hp in range(6):
                for k4 in range(4):
                    psq = psAB.tile([CH, 128], F32, tag="psA", name="psA", bufs=3)
                    nc.tensor.transpose(
                        psq[:], qT[hp][:, k4 * CH:(k4 + 1) * CH], ident[:])
                    st = stg_pool.tile([CH, 128], F32, tag="stg", name="stg")
                    pcopy(k4, st[:], psq[:])
                    nc.sync.dma_start(
                        q_nat[k4 * CH:(k4 + 1) * CH, hp * 128:(hp + 1) * 128],
                        st[:])
                    psk = psAB.tile([CH, 128], F32, tag="psA", name="psA", bufs=3)
                    nc.tensor.transpose(
                        psk[:], kT[hp][:, k4 * CH:(k4 + 1) * CH], ident[:])
                    st2 = stg_pool.tile([CH, 128], F32, tag="stg", name="stg")
                    pcopy(k4 + 1, st2[:], psk[:])
                    nc.sync.dma_start(
                        k_nat[k4 * CH:(k4 + 1) * CH, hp * 128:(hp + 1) * 128],
                        st2[:])

        # ---------------- phase C: stage-1 attention -----------------------
        es_xo = ExitStack()                   # xoT lives through phase E1
        xoT_pool = es_xo.enter_context(tc.tile_pool(name="xoT", bufs=1, side="right"))
        xoT = [xoT_pool.tile([128, FR * SQ], F32, tag=f"xoT{j}", name=f"xoT{j}")
               for j in range(6)]
        with tc.tile_pool(name="psC", bufs=1, space="PSUM") as psC, \
             tc.tile_pool(name="expp", bufs=2) as exp_pool, \
             tc.tile_pool(name="recipp", bufs=2) as recip_pool:
            for hp in range(6):
                for f in range(FR):
                    exp_sb = [
                        exp_pool.tile([CH, 2 * SQ], F32, tag="exp0", name="exp0"),
                        exp_pool.tile([CH, 2 * SQ], F32, tag="exp1", name="exp1")]
                    for e in range(2):
                        for kc in range(2):
                            sc = psC.tile([CH, SQ], F32, tag="sc", name="sc",
                                          bufs=4)
                            nc.tensor.matmul(
                                sc[:],
                                _mm_cast(kT[hp][64 * e:64 * e + 64,
                                                f * P + kc * CH:
                                                f * P + (kc + 1) * CH]),
                                _mm_cast(qT[hp][64 * e:64 * e + 64, :]),
                                start=True, stop=True,
                                tile_position=(64 * e, 0))
                            nc.scalar.activation(
                                exp_sb[e][:, kc * SQ:(kc + 1) * SQ], sc[:],
                                EXP, scale=SCALE)
                    sums_ps = psC.tile([128, SQ], F32, tag="sums", name="sums",
                                       bufs=2)
                    xo_ps = psC.tile([128, SQ], F32, tag="xo", name="xo", bufs=2)
                    for e in range(2):
                        for kc in range(2):
                            nc.tensor.matmul(
                                sums_ps[64 * e:64 * e + 64, :],
                                _mm_cast(ones_t[:, 0:64]),
                                _mm_cast(exp_sb[e][:, kc * SQ:(kc + 1) * SQ]),
                                start=kc == 0, stop=kc == 1,
                                tile_position=(0, 64 * e))
                        for kc in range(2):
                            nc.tensor.matmul(
                                xo_ps[64 * e:64 * e + 64, :],
                                _mm_cast(v_t[2 * f + kc][:,
                                         (2 * hp + e) * 64:(2 * hp + e) * 64 + 64]),
                                _mm_cast(exp_sb[e][:, kc * SQ:(kc + 1) * SQ]),
                                start=kc == 0, stop=kc == 1,
                                tile_position=(0, 64 * e))
                    recip = recip_pool.tile([128, SQ], F32, tag="recip",
                                            name="recip")
                    nc.vector.reciprocal(recip[:], sums_ps[:])
                    nc.vector.tensor_mul(
                        xoT[hp][:, f * SQ:(f + 1) * SQ], xo_ps[:], recip[:])

        es_bc.close()  # free kT / qT / v

        # ---------------- phase D: q2 projection ---------------------------
        es_q2 = ExitStack()                   # q2 lives through phase E1
        q2_pool = es_q2.enter_context(tc.tile_pool(name="q2", bufs=1, side="right"))
        q2_sb = [q2_pool.tile([CH, C], F32, tag=f"q2_{k}", name=f"q2_{k}")
                 for k in range(4)]
        with tc.tile_pool(name="wD", bufs=1) as wD_pool, \
             tc.tile_pool(name="psD", bufs=4, space="PSUM") as psD:
            wq2 = [wD_pool.tile([128, C], F32, tag=f"wq2_{ci}", name=f"wq2_{ci}")
                   for ci in range(6)]
            for ci in range(6):
                nc.sync.dma_start(wq2[ci][:], wqT[ci * 128:(ci + 1) * 128, :])
            for k4 in range(4):
                off = 392 * (k4 // 2) + CH * k4   # diagonal gather offset
                for ns in range(2):
                    ps = psD.tile([CH, 384], F32, tag="psD", name="psD")
                    for ci in range(6):
                        nc.tensor.matmul(
                            ps[:], _mm_cast(xoT[ci][:, off:off + CH]),
                            _mm_cast(wq2[ci][:, ns * 384:(ns + 1) * 384]),
                            start=ci == 0, stop=ci == 5)
                    pcopy(k4 + ns, q2_sb[k4][:, ns * 384:(ns + 1) * 384], ps[:])

        # ---------------- phase E1: kv projection + stage-2 mix -------------
        es_acc = ExitStack()                  # acc tiles live into phase E2
        acc_pool = es_acc.enter_context(tc.tile_pool(name="accp", bufs=1))
        acc_t = [acc_pool.tile([CH, C], F32, tag=f"acc{k}", name=f"acc{k}")
                 for k in range(4)]
        with tc.tile_pool(name="wE", bufs=1) as wE_pool, \
             tc.tile_pool(name="kvp", bufs=1) as kv_pool, \
             tc.tile_pool(name="psE", bufs=4, space="PSUM") as psE, \
             tc.tile_pool(name="s2p", bufs=2) as s2_pool, \
             tc.tile_pool(name="tmpp", bufs=2) as tmp_pool:
            wkv = [wE_pool.tile([128, 2 * C], F32, tag=f"wkv{ci}",
                                name=f"wkv{ci}") for ci in range(6)]
            for ci in range(6):
                nc.sync.dma_start(wkv[ci][:], wkvT[ci * 128:(ci + 1) * 128, :])

            for k4 in range(4):
                kv_half = []
                for fh in range(2):
                    kv_sb = kv_pool.tile([CH, 4 * 2 * C], F32, tag=f"kv{fh}",
                                         name=f"kv{fh}")
                    kv_half.append(kv_sb)
                    for fi in range(4):
                        f = fh * 4 + fi
                        for ns in range(3):
                            ps = psE.tile([CH, 512], F32, tag="psE", name="psE")
                            for ci in range(6):
                                nc.tensor.matmul(
                                    ps[:],
                                    _mm_cast(xoT[ci][:, f * SQ + k4 * CH:
                                                     f * SQ + (k4 + 1) * CH]),
                                    _mm_cast(wkv[ci][:, ns * 512:(ns + 1) * 512]),
                                    start=ci == 0, stop=ci == 5)
                            pcopy(fi + ns, kv_sb[:, fi * 1536 + ns * 512:
                                                 fi * 1536 + (ns + 1) * 512],
                                  ps[:])

                def kvap(f, lo, hi):
                    t = kv_half[f // 4]
                    fi = f % 4
                    return t[:, fi * 1536 + lo:fi * 1536 + hi]

                # stage-2 scores: s2[s, h, f] = sum_d q2*k2
                s2 = s2_pool.tile([CH, H * FR], F32, tag="s2", name="s2")
                s2v = s2[:].rearrange("p (h f) -> p h f", f=FR)
                for f in range(FR):
                    tmp = tmp_pool.tile([CH, C], F32, tag="tmp", name="tmp")
                    nc.vector.tensor_mul(tmp[:], kvap(f, 0, C), q2_sb[k4][:])
                    nc.vector.reduce_sum(
                        s2v[:, :, f], tmp[:].rearrange("p (h d) -> p h d", d=D),
                        axis=mybir.AxisListType.X)
                # softmax over f (SCALE folded into exp)
                e2 = s2_pool.tile([CH, H * FR], F32, tag="e2", name="e2")
                nc.scalar.activation(e2[:], s2[:], EXP, scale=SCALE)
                sum2 = s2_pool.tile([CH, H], F32, tag="sum2", name="sum2")
                nc.vector.reduce_sum(
                    sum2[:], e2[:].rearrange("p (h f) -> p h f", f=FR),
                    axis=mybir.AxisListType.X)
                rec2 = s2_pool.tile([CH, H], F32, tag="rec2", name="rec2")
                nc.vector.reciprocal(rec2[:], sum2[:])
                at2 = s2_pool.tile([CH, H * FR], F32, tag="at2", name="at2")
                at2v = at2[:].rearrange("p (h f) -> p h f", f=FR)
                nc.vector.tensor_mul(
                    at2v, e2[:].rearrange("p (h f) -> p h f", f=FR),
                    rec2[:].broadcast_to([CH, H, FR]))
                nc.sync.dma_start(attn_o[k4 * CH:(k4 + 1) * CH, :], at2[:])
                # weighted sum over f
                acc = acc_t[k4]
                accv = acc[:].rearrange("p (h d) -> p h d", d=D)
                for f in range(FR):
                    if f == 0:
                        nc.vector.tensor_mul(
                            accv,
                            kvap(f, C, 2 * C).rearrange("p (h d) -> p h d", d=D),
                            at2v[:, :, f].broadcast_to([CH, H, D]))
                    else:
                        tmp = tmp_pool.tile([CH, C], F32, tag="tmp", name="tmp")
                        nc.vector.tensor_mul(
                            tmp[:].rearrange("p (h d) -> p h d", d=D),
                            kvap(f, C, 2 * C).rearrange("p (h d) -> p h d", d=D),
                            at2v[:, :, f].broadcast_to([CH, H, D]))
                        nc.vector.tensor_add(acc[:], acc[:], tmp[:])

        es_q2.close()
        es_xo.close()

        # ---------------- phase E2: output projection ----------------------
        with tc.tile_pool(name="wP", bufs=1) as wP_pool, \
             tc.tile_pool(name="opt", bufs=2) as opT_pool, \
             tc.tile_pool(name="ostp", bufs=2) as ost_pool, \
             tc.tile_pool(name="psP", bufs=1, space="PSUM") as psP:
            wpj = [wP_pool.tile([128, C], F32, tag=f"wpj{ci}", name=f"wpj{ci}")
                   for ci in range(6)]
            for ci in range(6):
                nc.sync.dma_start(wpj[ci][:], wprojT[ci * 128:(ci + 1) * 128, :])
            for k4 in range(4):
                acc = acc_t[k4]
                opT = []
                for ci in range(6):
                    pst = psP.tile([128, CH], F32, tag="psT", name="psT", bufs=3)
                    nc.tensor.transpose(
                        pst[:], acc[:, ci * 128:(ci + 1) * 128], ident[:CH, :CH])
                    ot = opT_pool.tile([128, CH], F32, tag=f"opT{ci}",
                                       name=f"opT{ci}")
                    pcopy(ci, ot[:], pst[:])
                    opT.append(ot)
                ost = ost_pool.tile([CH, C], F32, tag="ost", name="ost")
                for ns in range(2):
                    ps = psP.tile([CH, 384], F32, tag="psE2", name="psE2", bufs=2)
                    for ci in range(6):
                        nc.tensor.matmul(
                            ps[:], _mm_cast(opT[ci][:]),
                            _mm_cast(wpj[ci][:, ns * 384:(ns + 1) * 384]),
                            start=ci == 0, stop=False)
                    nc.tensor.matmul(
                        ps[:], _mm_cast(ones_t[0:1, 0:CH]),
                        _mm_cast(bias_sb[:, ns * 384:(ns + 1) * 384]),
                        start=False, stop=True)
                    pcopy(ns, ost[:, ns * 384:(ns + 1) * 384], ps[:])
                nc.sync.dma_start(out_o[k4 * CH:(k4 + 1) * CH, :], ost[:])
        es_acc.close()

    _split_excess_waits(nc)
    return nc


# ------------------------------------------------------------- host driver
_nc_cache = {}


def _get_nc():
    if "nc" not in _nc_cache:
        _nc_cache["nc"] = _build()
    return _nc_cache["nc"]


def _perm_for(g):
    own = [2 * g, 2 * g + 1]
    return own + [f for f in range(FR) if f not in own]


def _prep_inputs(x, W_qkv, W_q, W_kv, W_proj, b_proj):
    x = np.asarray(x, np.float32).reshape(B, FR, P, C)
    wqkvT = np.ascontiguousarray(np.asarray(W_qkv, np.float32).T)
    wqT = np.ascontiguousarray(np.asarray(W_q, np.float32).T)
    wkvT = np.ascontiguousarray(np.asarray(W_kv, np.float32).T)
    wprojT = np.ascontiguousarray(np.asarray(W_proj, np.float32).T)
    bp = np.asarray(b_proj, np.float32).reshape(1, C)
    in_maps = []
    for core in range(8):
        b, g = divmod(core, 4)
        perm = _perm_for(g)
        xp = np.ascontiguousarray(x[b, perm].reshape(N, C))
        in_maps.append({
            "x": xp, "wqkvT": wqkvT, "wqT": wqT, "wkvT": wkvT,
            "wprojT": wprojT, "bproj": bp,
        })
    return in_maps


def _assemble(results):
    out = np.zeros((B * FR, P, C), np.float32)
    attn = np.zeros((B, H, N, FR), np.float32)
    q = np.zeros((B * FR, H, P, D), np.float32)
    k = np.zeros((B * FR, H, P, D), np.float32)
    v = np.zeros((B * FR, H, P, D), np.float32)
    for core in range(8):
        b, g = divmod(core, 4)
        perm = _perm_for(g)
        r = results[core]
        for j in range(2):                      # own frames (perm order 0,1)
            f = 2 * g + j
            rows = slice(j * P, (j + 1) * P)
            out[b * FR + f] = r["out_o"][rows]
            q[b * FR + f] = r["q_nat"][rows].reshape(P, H, D).transpose(1, 0, 2)
            k[b * FR + f] = r["k_nat"][rows].reshape(P, H, D).transpose(1, 0, 2)
            v[b * FR + f] = r["v_nat"][rows].reshape(P, H, D).transpose(1, 0, 2)
        a = r["attn_o"].reshape(SQ, H, FR)      # [s, h, f_perm]
        s0 = SQ * g
        for j, f in enumerate(perm):
            attn[b, :, s0:s0 + SQ, f] = a[:, :, j].T
    qkv = np.stack([q, k, v])
    return out, attn, qkv


def _run(in_maps, trace=False):
    from concourse.bass_utils import run_bass_kernel_spmd
    nc = _get_nc()
    res = run_bass_kernel_spmd(nc, in_maps, list(range(8)), trace=trace)
    return res


def kernel(x, W_qkv, W_q, W_kv, W_proj, b_proj, seq_len, num_frames):
    assert int(seq_len) == P and int(num_frames) == FR
    in_maps = _prep_inputs(x, W_qkv, W_q, W_kv, W_proj, b_proj)
    res = _run(in_maps, trace=False)
    return _assemble(res.results)


# ------------------------------------------------- steady-state benchmarking
def make_bench(in_maps, n_cores=8):
    """Build a jitted sharded executable (no donation) + device-resident
    inputs.  Returns (run_once, results_fn): run_once() executes and blocks;
    results_fn() returns per-core output dicts."""
    import jax
    from concourse import bass2jax
    from jax.experimental.shard_map import shard_map
    from jax.sharding import Mesh, PartitionSpec, NamedSharding

    nc = _get_nc()
    bass2jax.install_neuronx_cc_hook()
    partition_name = (nc.partition_id_tensor.name
                      if nc.partition_id_tensor else None)
    in_names, out_names, out_avals, zero_outs = [], [], [], []
    for alloc in nc.m.functions[0].allocations:
        if not isinstance(alloc, mybir.MemoryLocationSet):
            continue
        name = alloc.memorylocations[0].name
        if alloc.kind == "ExternalInput":
            if name != partition_name:
                in_names.append(name)
        elif alloc.kind == "ExternalOutput":
            out_names.append(name)
            shape = tuple(alloc.tensor_shape)
            dtype = mybir.dt.np(alloc.dtype)
            out_avals.append(jax.core.ShapedArray(shape, dtype))
            zero_outs.append(np.zeros(shape, dtype))
    n_params = len(in_names)
    all_in_names = in_names + out_names + (
        [partition_name] if partition_name else [])

    def _body(*args):
        operands = list(args)
        if partition_name is not None:
            operands.append(bass2jax.partition_id_tensor())
        outs = bass2jax._bass_exec_p.bind(
            *operands,
            out_avals=tuple(out_avals),
            in_names=tuple(all_in_names),
            out_names=tuple(out_names),
            lowering_input_output_aliases=(),
            sim_require_finite=True,
            sim_require_nnan=True,
            nc=nc,
        )
        return tuple(outs)

    devices = jax.devices()[:n_cores]
    mesh = Mesh(np.asarray(devices), ("core",))
    in_specs = (PartitionSpec("core"),) * (n_params + len(out_names))
    out_specs = (PartitionSpec("core"),) * len(out_names)
    fn = jax.jit(shard_map(_body, mesh=mesh, in_specs=in_specs,
                           out_specs=out_specs, check_rep=False),
                 keep_unused=True)
    sh = NamedSharding(mesh, PartitionSpec("core"))
    dev_in = [
        jax.device_put(
            np.concatenate([np.asarray(in_maps[c][nm]) for c in range(n_cores)],
                           axis=0), sh)
        for nm in in_names
    ]
    dev_zero = [
        jax.device_put(
            np.zeros((n_cores * z.shape[0], *z.shape[1:]), z.dtype), sh)
        for z in zero_outs
    ]
    state = {}

    def run_once():
        out = fn(*dev_in, *dev_zero)
        jax.block_until_ready(out)
        state["out"] = out
        return out

    def results_fn():
        out = state["out"]
        return [
            {nm: np.asarray(out[i]).reshape(n_cores, *out_avals[i].shape)[c]
             for i, nm in enumerate(out_names)}
            for c in range(n_cores)
        ]

    return run_once, results_fn
